# revision 6
# baseline (speedup 1.0000x reference)
"""Trainium2 Bass kernel: BN(eval) -> sign -> Conv1d(K=7,pad=3) -> alpha -> PReLU -> MaxPool2.

Strategy (hardcoded for B=64, CIN=64, L=4096, COUT=128, K=7):
  - Data-parallel over batch: 8 samples per NeuronCore x 8 cores.
  - Host folds BN into per-channel (scale, bias); ScalarE computes
    sign(I*scale+bias) for a PAIR of samples at once ([128, 4096] -> bf16).
  - Host folds alpha into conv weights; conv done as 4 PSUM-accumulated
    bf16 matmuls per 512-col output tile: k-pairs (0,1),(2,3),(4,5),(6,-)
    packed into a 128-row contraction, with the input duplicated into
    partitions 64..127 shifted by one column.
  - MaxPool(2) straight out of PSUM via DVE tensor_reduce(max) over
    [128, 256, 2] views; PReLU(a) = a*x + relu((1-a)*x) via one ScalarE
    relu + one GpSimd scalar_tensor_tensor.
"""

import json
import sys

for _p in ("/opt/trn_rl_repo", "/root/.axon_site/_ro/trn_rl_repo"):
    if _p not in sys.path:
        sys.path.append(_p)

import numpy as np
import ml_dtypes

import concourse.bass as bass
import concourse.tile as tile
from concourse import mybir
from concourse.bass_utils import run_bass_kernel_spmd

B, CIN, L, COUT, K = 64, 64, 4096, 128, 7
PAD = 3
BN_EPS = 1e-5
N_CORES = 8
BPC = B // N_CORES  # samples per core
LOUT = L // 2       # 2048 pooled length
NT = L // 512       # 8 output tiles of 512 cols
XW = L + 8          # padded X2 width (4104)

_CACHE: dict = {}


def build_program() -> "bass.Bass":
    nc = bass.Bass(trn_type="TRN2")
    I8 = nc.dram_tensor("I8", [BPC, CIN, L], mybir.dt.float32, kind="ExternalInput")
    W = nc.dram_tensor("W", [128, 4 * 128], mybir.dt.bfloat16, kind="ExternalInput")
    SBp = nc.dram_tensor("SBp", [128, 4], mybir.dt.float32, kind="ExternalInput")
    O8 = nc.dram_tensor("O8", [BPC, COUT, LOUT], mybir.dt.float32, kind="ExternalOutput")

    iflat = I8.ap().flatten_outer_dims()  # [BPC*64, 4096]
    oflat = O8.ap().flatten_outer_dims()  # [BPC*128, 2048]

    AF = mybir.ActivationFunctionType
    with tile.TileContext(nc) as tc:
        with (
            tc.tile_pool(name="consts", bufs=1) as consts,
            tc.tile_pool(name="ipair", bufs=2) as ipool,
            tc.tile_pool(name="sgn", bufs=2) as spool,
            tc.tile_pool(name="x2", bufs=3) as xpool,
            tc.tile_pool(name="pooled", bufs=2) as plpool,
            tc.tile_pool(name="relu", bufs=2) as rpool,
            tc.tile_pool(name="outp", bufs=2) as opool,
            tc.tile_pool(name="ps", bufs=8, space="PSUM") as pspool,
        ):
            w_sb = consts.tile([128, 4 * 128], mybir.dt.bfloat16)
            nc.sync.dma_start(w_sb[:], W.ap()[:])
            sb_sb = consts.tile([128, 4], mybir.dt.float32)
            nc.sync.dma_start(sb_sb[:], SBp.ap()[:])
            sgn_scale = sb_sb[:, 0:1]
            sgn_bias = sb_sb[:, 1:2]
            relu_scale = sb_sb[:, 2:3]  # 1 - a
            slope = sb_sb[:, 3:4]       # a

            for t in range(BPC // 2):
                ip = ipool.tile([128, L], mybir.dt.float32)
                nc.sync.dma_start(ip[:], iflat[128 * t : 128 * (t + 1), :])
                # sign of BN affine for two samples at once
                sg = spool.tile([128, L], mybir.dt.bfloat16)
                nc.scalar.activation(sg[:], ip[:], AF.Sign, bias=sgn_bias, scale=sgn_scale)

                for h in range(2):
                    b = 2 * t + h
                    sgh = sg[64 * h : 64 * h + 64, :]
                    # X2: rows 0-63 = xpad (data at cols 3..L+2),
                    #     rows 64-127 = xpad shifted +1 (data at cols 2..L+1)
                    x2 = xpool.tile([128, XW], mybir.dt.bfloat16)
                    nc.gpsimd.memset(x2[0:64, 0:3], 0.0)
                    nc.gpsimd.memset(x2[0:64, L + 3 : XW], 0.0)
                    nc.gpsimd.memset(x2[64:128, 0:2], 0.0)
                    nc.gpsimd.memset(x2[64:128, L + 2 : XW], 0.0)
                    if h == 0:
                        nc.gpsimd.tensor_copy(x2[0:64, 3 : L + 3], sgh)
                        nc.vector.tensor_copy(x2[64:128, 2 : L + 2], sgh)
                    else:
                        nc.vector.tensor_copy(x2[0:64, 3 : L + 3], sgh)
                        nc.gpsimd.tensor_copy(x2[64:128, 2 : L + 2], sgh)

                    pss = [
                        pspool.tile([128, 512], mybir.dt.float32, name=f"psb{i}", tag="psb")
                        for i in range(NT)
                    ]
                    for p in range(4):
                        lhsT = w_sb[:, 128 * p : 128 * (p + 1)]
                        for i in range(NT):
                            nc.tensor.matmul(
                                pss[i][:],
                                lhsT,
                                x2[:, 512 * i + 2 * p : 512 * i + 2 * p + 512],
                                start=(p == 0),
                                stop=(p == 3),
                            )

                    pl = plpool.tile([128, LOUT], mybir.dt.float32)
                    for i in range(NT):
                        nc.vector.tensor_reduce(
                            pl[:, 256 * i : 256 * (i + 1)],
                            pss[i][:].rearrange("p (n two) -> p n two", two=2),
                            mybir.AxisListType.X,
                            mybir.AluOpType.max,
                        )
                    r = rpool.tile([128, LOUT], mybir.dt.float32)
                    nc.scalar.activation(r[:], pl[:], AF.Relu, scale=relu_scale)
                    r2 = rpool.tile([128, LOUT], mybir.dt.float32)
                    nc.scalar.activation(r2[:], pl[:], AF.Copy, scale=slope)
                    o = opool.tile([128, LOUT], mybir.dt.float32)
                    nc.gpsimd.tensor_add(o[:], r[:], r2[:])
                    nc.sync.dma_start(oflat[128 * b : 128 * (b + 1), :], o[:])
    return nc


def _split_sync_waits_json(bir: bytes) -> bytes:
    """Walrus in this toolchain accepts at most one sync-wait per instruction.
    Hoist multi-wait sync_info lists into preceding single-wait EventSemaphore
    instructions on the same engine queue (the same form engine.wait_ge()
    lowers to), preserving program order and on_update placement."""
    j = json.loads(bir)
    n_split = 0
    for fn in j.get("functions", []):
        for blk in fn.get("blocks", []):
            ins_list = blk.get("instructions")
            if not ins_list:
                continue
            out = []
            for ins in ins_list:
                si = ins.get("sync_info")
                waits = si.get("on_wait") if si else None
                if waits and len(waits) > 1:
                    for i, w in enumerate(waits):
                        out.append(
                            {
                                "debug": ins.get("debug", 0),
                                "engine": ins["engine"],
                                "ins": [],
                                "outs": [],
                                "name": f"{ins['name']}-antw{i}",
                                "opcode": "EventSemaphore",
                                "sync_info": {"on_update": [], "on_wait": [w]},
                            }
                        )
                    si["on_wait"] = []
                    n_split += 1
                out.append(ins)
            blk["instructions"] = out
    return json.dumps(j).encode()


def get_program() -> "bass.Bass":
    if "nc" not in _CACHE:
        nc = build_program()
        orig = nc.to_json_bytes
        nc.to_json_bytes = lambda: _split_sync_waits_json(orig())
        _CACHE["nc"] = nc
    return _CACHE["nc"]


def prep_inputs(I, bn_gamma, bn_beta, bn_mean, bn_var, conv_w, alpha, prelu_w):
    """Host-side folding: BN -> (scale, bias); alpha -> weights; k-pair lhsT packing."""
    f32 = np.float32
    gamma = np.asarray(bn_gamma, f32)
    beta = np.asarray(bn_beta, f32)
    mean = np.asarray(bn_mean, f32)
    var = np.asarray(bn_var, f32)
    s = gamma / np.sqrt(var + f32(BN_EPS))        # [CIN]
    t = beta - mean * s                            # [CIN]

    w = np.asarray(conv_w, f32) * np.asarray(alpha, f32)[:, None, None]  # [COUT, CIN, K]
    Wb = np.zeros((128, 4 * 128), np.float32)
    for p in range(4):
        k0, k1 = 2 * p, 2 * p + 1
        Wb[0:64, 128 * p : 128 * p + 128] = w[:, :, k0].T
        if k1 < K:
            Wb[64:128, 128 * p : 128 * p + 128] = w[:, :, k1].T
    Wb = Wb.astype(ml_dtypes.bfloat16)

    a = f32(np.asarray(prelu_w, f32).reshape(-1)[0])
    sbp = np.zeros((128, 4), f32)
    sbp[0:64, 0] = s
    sbp[64:128, 0] = s
    sbp[0:64, 1] = t
    sbp[64:128, 1] = t
    sbp[:, 2] = f32(1.0) - a
    sbp[:, 3] = a
    return Wb, sbp


def kernel(I, bn_gamma, bn_beta, bn_mean, bn_var, conv_w, alpha, prelu_w):
    I = np.ascontiguousarray(np.asarray(I, np.float32))
    assert I.shape == (B, CIN, L), I.shape
    Wb, sbp = prep_inputs(I, bn_gamma, bn_beta, bn_mean, bn_var, conv_w, alpha, prelu_w)

    nc = get_program()
    in_maps = [
        {"I8": I[BPC * c : BPC * (c + 1)], "W": Wb, "SBp": sbp} for c in range(N_CORES)
    ]
    res = run_bass_kernel_spmd(nc, in_maps, core_ids=list(range(N_CORES)))
    out = np.concatenate([res.results[c]["O8"] for c in range(N_CORES)], axis=0)
    return np.ascontiguousarray(out.astype(np.float32))


# revision 9
# speedup vs baseline: 1.6005x; 1.6005x over previous
"""Trainium2 Bass kernel: BN(eval) -> sign -> Conv1d(K=7,pad=3) -> alpha -> PReLU -> MaxPool2.

Strategy (hardcoded for B=64, CIN=64, L=4096, COUT=128, K=7):
  - Data-parallel over batch: 8 samples per NeuronCore x 8 cores.
  - Host folds BN into per-channel (scale, bias); ScalarE computes
    sign(I*scale+bias) for a PAIR of samples at once ([128, 4096] -> bf16).
  - Host folds alpha into conv weights; conv done as 4 PSUM-accumulated
    bf16 matmuls per 512-col output tile: k-pairs (0,1),(2,3),(4,5),(6,-)
    packed into a 128-row contraction, with the input duplicated into
    partitions 64..127 shifted by one column.
  - MaxPool(2) straight out of PSUM via DVE tensor_reduce(max) over
    [128, 256, 2] views; PReLU(a) = a*x + relu((1-a)*x) via one ScalarE
    relu + one GpSimd scalar_tensor_tensor.
"""

import json
import sys

for _p in ("/opt/trn_rl_repo", "/root/.axon_site/_ro/trn_rl_repo"):
    if _p not in sys.path:
        sys.path.append(_p)

import numpy as np
import ml_dtypes

import concourse.bass as bass
import concourse.tile as tile
from concourse import mybir
from concourse.bass_utils import run_bass_kernel_spmd

B, CIN, L, COUT, K = 64, 64, 4096, 128, 7
PAD = 3
BN_EPS = 1e-5
N_CORES = 8
BPC = B // N_CORES  # samples per core
LOUT = L // 2       # 2048 pooled length
NT = L // 512       # 8 output tiles of 512 cols
XW = L + 8          # padded X2 width (4104)

_CACHE: dict = {}


def build_program() -> "bass.Bass":
    nc = bass.Bass(trn_type="TRN2")
    I8 = nc.dram_tensor("I8", [BPC, CIN, L], mybir.dt.float32, kind="ExternalInput")
    W = nc.dram_tensor("W", [128, 4 * 128], mybir.dt.bfloat16, kind="ExternalInput")
    SBp = nc.dram_tensor("SBp", [128, 4], mybir.dt.float32, kind="ExternalInput")
    O8 = nc.dram_tensor("O8", [BPC, COUT, LOUT], mybir.dt.float32, kind="ExternalOutput")

    iflat = I8.ap().flatten_outer_dims()  # [BPC*64, 4096]
    oflat = O8.ap().flatten_outer_dims()  # [BPC*128, 2048]

    AF = mybir.ActivationFunctionType
    with tile.TileContext(nc) as tc:
        with (
            tc.tile_pool(name="consts", bufs=1) as consts,
            tc.tile_pool(name="ipair", bufs=2) as ipool,
            tc.tile_pool(name="sgn", bufs=2) as spool,
            tc.tile_pool(name="x2", bufs=3) as xpool,
            tc.tile_pool(name="pooled", bufs=2) as plpool,
            tc.tile_pool(name="outp", bufs=2) as opool,
            tc.tile_pool(name="ps", bufs=8, space="PSUM") as pspool,
        ):
            w_sb = consts.tile([128, 4 * 128], mybir.dt.bfloat16)
            nc.sync.dma_start(w_sb[:], W.ap()[:])
            sb_sb = consts.tile([128, 4], mybir.dt.float32)
            nc.sync.dma_start(sb_sb[:], SBp.ap()[:])
            sgn_scale = sb_sb[:, 0:1]
            sgn_bias = sb_sb[:, 1:2]
            relu_scale = sb_sb[:, 2:3]  # 1 - a
            slope = sb_sb[:, 3:4]       # a

            for t in range(BPC // 2):
                ip = ipool.tile([128, L], mybir.dt.float32)
                nc.sync.dma_start(ip[:], iflat[128 * t : 128 * (t + 1), :])
                # sign of BN affine for two samples at once
                sg = spool.tile([128, L], mybir.dt.bfloat16)
                nc.scalar.activation(sg[:], ip[:], AF.Sign, bias=sgn_bias, scale=sgn_scale)

                for h in range(2):
                    b = 2 * t + h
                    sgh = sg[64 * h : 64 * h + 64, :]
                    # X2: rows 0-63 = xpad (data at cols 3..L+2),
                    #     rows 64-127 = xpad shifted +1 (data at cols 2..L+1)
                    x2 = xpool.tile([128, XW], mybir.dt.bfloat16)
                    nc.gpsimd.memset(x2[0:64, 0:3], 0.0)
                    nc.gpsimd.memset(x2[0:64, L + 3 : XW], 0.0)
                    nc.gpsimd.memset(x2[64:128, 0:2], 0.0)
                    nc.gpsimd.memset(x2[64:128, L + 2 : XW], 0.0)
                    # duplicate the sign row-block into both halves (bottom
                    # shifted one column) via SBUF->SBUF DMA on the ACT ring
                    nc.scalar.dma_start(x2[0:64, 3 : L + 3], sgh)
                    nc.scalar.dma_start(x2[64:128, 2 : L + 2], sgh)

                    pss = [
                        pspool.tile([128, 512], mybir.dt.float32, name=f"psb{i}", tag="psb")
                        for i in range(NT)
                    ]
                    for p in range(4):
                        lhsT = w_sb[:, 128 * p : 128 * (p + 1)]
                        for i in range(NT):
                            nc.tensor.matmul(
                                pss[i][:],
                                lhsT,
                                x2[:, 512 * i + 2 * p : 512 * i + 2 * p + 512],
                                start=(p == 0),
                                stop=(p == 3),
                            )

                    pl = plpool.tile([128, LOUT], mybir.dt.float32)
                    for i in range(NT):
                        nc.vector.tensor_reduce(
                            pl[:, 256 * i : 256 * (i + 1)],
                            pss[i][:].rearrange("p (n two) -> p n two", two=2),
                            mybir.AxisListType.X,
                            mybir.AluOpType.max,
                        )
                    # prelu(x) = max(a*x, x) for 0 <= a <= 1
                    o = opool.tile([128, LOUT], mybir.dt.float32)
                    nc.vector.scalar_tensor_tensor(
                        o[:], pl[:], slope, pl[:],
                        mybir.AluOpType.mult, mybir.AluOpType.max,
                    )
                    nc.sync.dma_start(oflat[128 * b : 128 * (b + 1), :], o[:])
    return nc


def _split_sync_waits_json(bir: bytes) -> bytes:
    """Walrus in this toolchain accepts at most one sync-wait per instruction.
    Hoist multi-wait sync_info lists into preceding single-wait EventSemaphore
    instructions on the same engine queue (the same form engine.wait_ge()
    lowers to), preserving program order and on_update placement."""
    j = json.loads(bir)
    n_split = 0
    for fn in j.get("functions", []):
        for blk in fn.get("blocks", []):
            ins_list = blk.get("instructions")
            if not ins_list:
                continue
            out = []
            for ins in ins_list:
                si = ins.get("sync_info")
                waits = si.get("on_wait") if si else None
                if waits and len(waits) > 1:
                    for i, w in enumerate(waits):
                        out.append(
                            {
                                "debug": ins.get("debug", 0),
                                "engine": ins["engine"],
                                "ins": [],
                                "outs": [],
                                "name": f"{ins['name']}-antw{i}",
                                "opcode": "EventSemaphore",
                                "sync_info": {"on_update": [], "on_wait": [w]},
                            }
                        )
                    si["on_wait"] = []
                    n_split += 1
                out.append(ins)
            blk["instructions"] = out
    return json.dumps(j).encode()


def get_program() -> "bass.Bass":
    if "nc" not in _CACHE:
        nc = build_program()
        orig = nc.to_json_bytes
        nc.to_json_bytes = lambda: _split_sync_waits_json(orig())
        _CACHE["nc"] = nc
    return _CACHE["nc"]


def prep_inputs(I, bn_gamma, bn_beta, bn_mean, bn_var, conv_w, alpha, prelu_w):
    """Host-side folding: BN -> (scale, bias); alpha -> weights; k-pair lhsT packing."""
    f32 = np.float32
    gamma = np.asarray(bn_gamma, f32)
    beta = np.asarray(bn_beta, f32)
    mean = np.asarray(bn_mean, f32)
    var = np.asarray(bn_var, f32)
    s = gamma / np.sqrt(var + f32(BN_EPS))        # [CIN]
    t = beta - mean * s                            # [CIN]

    w = np.asarray(conv_w, f32) * np.asarray(alpha, f32)[:, None, None]  # [COUT, CIN, K]
    Wb = np.zeros((128, 4 * 128), np.float32)
    for p in range(4):
        k0, k1 = 2 * p, 2 * p + 1
        Wb[0:64, 128 * p : 128 * p + 128] = w[:, :, k0].T
        if k1 < K:
            Wb[64:128, 128 * p : 128 * p + 128] = w[:, :, k1].T
    Wb = Wb.astype(ml_dtypes.bfloat16)

    a = f32(np.asarray(prelu_w, f32).reshape(-1)[0])
    sbp = np.zeros((128, 4), f32)
    sbp[0:64, 0] = s
    sbp[64:128, 0] = s
    sbp[0:64, 1] = t
    sbp[64:128, 1] = t
    sbp[:, 2] = f32(1.0) - a
    sbp[:, 3] = a
    return Wb, sbp


def kernel(I, bn_gamma, bn_beta, bn_mean, bn_var, conv_w, alpha, prelu_w):
    I = np.ascontiguousarray(np.asarray(I, np.float32))
    assert I.shape == (B, CIN, L), I.shape
    Wb, sbp = prep_inputs(I, bn_gamma, bn_beta, bn_mean, bn_var, conv_w, alpha, prelu_w)

    nc = get_program()
    in_maps = [
        {"I8": I[BPC * c : BPC * (c + 1)], "W": Wb, "SBp": sbp} for c in range(N_CORES)
    ]
    res = run_bass_kernel_spmd(nc, in_maps, core_ids=list(range(N_CORES)))
    out = np.concatenate([res.results[c]["O8"] for c in range(N_CORES)], axis=0)
    return np.ascontiguousarray(out.astype(np.float32))


# revision 12
# speedup vs baseline: 1.7207x; 1.0751x over previous
"""Trainium2 Bass kernel: BN(eval) -> sign -> Conv1d(K=7,pad=3) -> alpha -> PReLU -> MaxPool2.

Strategy (hardcoded for B=64, CIN=64, L=4096, COUT=128, K=7):
  - Data-parallel over batch: 8 samples per NeuronCore x 8 cores.
  - Host folds BN into per-channel (scale, bias); ScalarE computes
    sign(I*scale+bias) for a PAIR of samples at once ([128, 4096] -> bf16).
  - Host folds alpha into conv weights; conv done as 4 PSUM-accumulated
    bf16 matmuls per 512-col output tile: k-pairs (0,1),(2,3),(4,5),(6,-)
    packed into a 128-row contraction, with the input duplicated into
    partitions 64..127 shifted by one column.
  - MaxPool(2) straight out of PSUM via DVE tensor_reduce(max) over
    [128, 256, 2] views; PReLU(a) = a*x + relu((1-a)*x) via one ScalarE
    relu + one GpSimd scalar_tensor_tensor.
"""

import json
import sys

for _p in ("/opt/trn_rl_repo", "/root/.axon_site/_ro/trn_rl_repo"):
    if _p not in sys.path:
        sys.path.append(_p)

import numpy as np
import ml_dtypes

import concourse.bass as bass
import concourse.tile as tile
from concourse import mybir
from concourse.bass_utils import run_bass_kernel_spmd

B, CIN, L, COUT, K = 64, 64, 4096, 128, 7
PAD = 3
BN_EPS = 1e-5
N_CORES = 8
BPC = B // N_CORES  # samples per core
LOUT = L // 2       # 2048 pooled length
NT = L // 512       # 8 output tiles of 512 cols
XW = L + 8          # padded X2 width (4104)

_CACHE: dict = {}


def build_program() -> "bass.Bass":
    nc = bass.Bass(trn_type="TRN2")
    I8 = nc.dram_tensor("I8", [BPC, CIN, L], mybir.dt.float32, kind="ExternalInput")
    W = nc.dram_tensor("W", [128, K * 128], mybir.dt.bfloat16, kind="ExternalInput")
    SBp = nc.dram_tensor("SBp", [128, 4], mybir.dt.float32, kind="ExternalInput")
    O8 = nc.dram_tensor("O8", [BPC, COUT, LOUT], mybir.dt.bfloat16, kind="ExternalOutput")

    iflat = I8.ap().flatten_outer_dims()  # [BPC*64, 4096]
    oflat = O8.ap().flatten_outer_dims()  # [BPC*128, 2048]

    AF = mybir.ActivationFunctionType
    SGW = L + 8  # sg width: cols 0-2 zero pad, 3..L+2 data, L+3.. zero
    NHALF = NT // 2  # 4 l-tiles per half (A half + B half = 8 PSUM banks)
    with tile.TileContext(nc) as tc:
        with (
            tc.tile_pool(name="consts", bufs=1) as consts,
            tc.tile_pool(name="ipair", bufs=2) as ipool,
            tc.tile_pool(name="sgn", bufs=2) as spool,
            tc.tile_pool(name="pooled", bufs=2) as plpool,
            tc.tile_pool(name="outp", bufs=2) as opool,
            tc.tile_pool(name="ps", bufs=8, space="PSUM") as pspool,
        ):
            w_sb = consts.tile([128, K * 128], mybir.dt.bfloat16)
            nc.sync.dma_start(w_sb[:], W.ap()[:])
            sb_sb = consts.tile([128, 4], mybir.dt.float32)
            nc.sync.dma_start(sb_sb[:], SBp.ap()[:])
            sgn_scale = sb_sb[:, 0:1]
            sgn_bias = sb_sb[:, 1:2]
            slope = sb_sb[:, 3:4]  # a

            for t in range(BPC // 2):
                ip = ipool.tile([128, L], mybir.dt.float32)
                nc.sync.dma_start(ip[:], iflat[128 * t : 128 * (t + 1), :])
                # sign of BN affine for two samples at once; sg doubles as the
                # padded conv input (rows 0-63 = sample A, 64-127 = sample B)
                sg = spool.tile([128, SGW], mybir.dt.bfloat16)
                nc.gpsimd.memset(sg[:, 0:3], 0.0)
                nc.gpsimd.memset(sg[:, L + 3 : SGW], 0.0)
                nc.scalar.activation(
                    sg[:, 3 : L + 3], ip[:], AF.Sign, bias=sgn_bias, scale=sgn_scale
                )

                pla = plpool.tile([128, LOUT], mybir.dt.bfloat16, name="pla", tag="pla")
                plb = plpool.tile([128, LOUT], mybir.dt.bfloat16, name="plb", tag="plb")
                for half in range(2):
                    psa = [
                        pspool.tile([128, 512], mybir.dt.float32, name=f"psa{i}", tag="psb")
                        for i in range(NHALF)
                    ]
                    psb = [
                        pspool.tile([128, 512], mybir.dt.float32, name=f"psb{i}", tag="psb")
                        for i in range(NHALF)
                    ]
                    for k in range(K):
                        lta = w_sb[0:64, 128 * k : 128 * (k + 1)]
                        ltb = w_sb[64:128, 128 * k : 128 * (k + 1)]
                        for i in range(NHALF):
                            c0 = 512 * (half * NHALF + i) + k
                            nc.tensor.matmul(
                                psa[i][:], lta, sg[0:64, c0 : c0 + 512],
                                start=(k == 0), stop=(k == K - 1),
                            )
                            nc.tensor.matmul(
                                psb[i][:], ltb, sg[64:128, c0 : c0 + 512],
                                start=(k == 0), stop=(k == K - 1),
                            )
                    for i in range(NHALF):
                        o0 = 256 * (half * NHALF + i)
                        nc.vector.tensor_reduce(
                            pla[:, o0 : o0 + 256],
                            psa[i][:].rearrange("p (n two) -> p n two", two=2),
                            mybir.AxisListType.X,
                            mybir.AluOpType.max,
                        )
                        nc.vector.tensor_reduce(
                            plb[:, o0 : o0 + 256],
                            psb[i][:].rearrange("p (n two) -> p n two", two=2),
                            mybir.AxisListType.X,
                            mybir.AluOpType.max,
                        )
                # prelu(x) = max(a*x, x) for 0 <= a <= 1
                for h, pl in ((0, pla), (1, plb)):
                    b = 2 * t + h
                    o = opool.tile([128, LOUT], mybir.dt.bfloat16)
                    nc.vector.scalar_tensor_tensor(
                        o[:], pl[:], slope, pl[:],
                        mybir.AluOpType.mult, mybir.AluOpType.max,
                    )
                    nc.sync.dma_start(oflat[128 * b : 128 * (b + 1), :], o[:])
    return nc


def _split_sync_waits_json(bir: bytes) -> bytes:
    """Walrus in this toolchain accepts at most one sync-wait per instruction.
    Hoist multi-wait sync_info lists into preceding single-wait EventSemaphore
    instructions on the same engine queue (the same form engine.wait_ge()
    lowers to), preserving program order and on_update placement."""
    j = json.loads(bir)
    n_split = 0
    for fn in j.get("functions", []):
        for blk in fn.get("blocks", []):
            ins_list = blk.get("instructions")
            if not ins_list:
                continue
            out = []
            for ins in ins_list:
                si = ins.get("sync_info")
                waits = si.get("on_wait") if si else None
                if waits and len(waits) > 1:
                    for i, w in enumerate(waits):
                        out.append(
                            {
                                "debug": ins.get("debug", 0),
                                "engine": ins["engine"],
                                "ins": [],
                                "outs": [],
                                "name": f"{ins['name']}-antw{i}",
                                "opcode": "EventSemaphore",
                                "sync_info": {"on_update": [], "on_wait": [w]},
                            }
                        )
                    si["on_wait"] = []
                    n_split += 1
                out.append(ins)
            blk["instructions"] = out
    return json.dumps(j).encode()


def get_program() -> "bass.Bass":
    if "nc" not in _CACHE:
        nc = build_program()
        orig = nc.to_json_bytes
        nc.to_json_bytes = lambda: _split_sync_waits_json(orig())
        _CACHE["nc"] = nc
    return _CACHE["nc"]


def prep_inputs(I, bn_gamma, bn_beta, bn_mean, bn_var, conv_w, alpha, prelu_w):
    """Host-side folding: BN -> (scale, bias); alpha -> weights; k-pair lhsT packing."""
    f32 = np.float32
    gamma = np.asarray(bn_gamma, f32)
    beta = np.asarray(bn_beta, f32)
    mean = np.asarray(bn_mean, f32)
    var = np.asarray(bn_var, f32)
    s = gamma / np.sqrt(var + f32(BN_EPS))        # [CIN]
    t = beta - mean * s                            # [CIN]

    w = np.asarray(conv_w, f32) * np.asarray(alpha, f32)[:, None, None]  # [COUT, CIN, K]
    Wb = np.zeros((128, K * 128), np.float32)
    for k in range(K):
        Wb[0:64, 128 * k : 128 * k + 128] = w[:, :, k].T
        Wb[64:128, 128 * k : 128 * k + 128] = w[:, :, k].T
    Wb = Wb.astype(ml_dtypes.bfloat16)

    a = f32(np.asarray(prelu_w, f32).reshape(-1)[0])
    sbp = np.zeros((128, 4), f32)
    sbp[0:64, 0] = s
    sbp[64:128, 0] = s
    sbp[0:64, 1] = t
    sbp[64:128, 1] = t
    sbp[:, 2] = f32(1.0) - a
    sbp[:, 3] = a
    return Wb, sbp


def kernel(I, bn_gamma, bn_beta, bn_mean, bn_var, conv_w, alpha, prelu_w):
    I = np.ascontiguousarray(np.asarray(I, np.float32))
    assert I.shape == (B, CIN, L), I.shape
    Wb, sbp = prep_inputs(I, bn_gamma, bn_beta, bn_mean, bn_var, conv_w, alpha, prelu_w)

    nc = get_program()
    in_maps = [
        {"I8": I[BPC * c : BPC * (c + 1)], "W": Wb, "SBp": sbp} for c in range(N_CORES)
    ]
    res = run_bass_kernel_spmd(nc, in_maps, core_ids=list(range(N_CORES)))
    out = np.concatenate(
        [np.asarray(res.results[c]["O8"]) for c in range(N_CORES)], axis=0
    )
    return np.ascontiguousarray(out.astype(np.float32))


# revision 13
# speedup vs baseline: 2.0607x; 1.1976x over previous
"""Trainium2 Bass kernel: BN(eval) -> sign -> Conv1d(K=7,pad=3) -> alpha -> PReLU -> MaxPool2.

Strategy (hardcoded for B=64, CIN=64, L=4096, COUT=128, K=7):
  - Data-parallel over batch: 8 samples per NeuronCore x 8 cores.
  - Host folds BN into per-channel (scale, bias); ScalarE computes
    sign(I*scale+bias) for a PAIR of samples at once ([128, 4096] -> bf16).
  - Host folds alpha into conv weights; conv done as 4 PSUM-accumulated
    bf16 matmuls per 512-col output tile: k-pairs (0,1),(2,3),(4,5),(6,-)
    packed into a 128-row contraction, with the input duplicated into
    partitions 64..127 shifted by one column.
  - MaxPool(2) straight out of PSUM via DVE tensor_reduce(max) over
    [128, 256, 2] views; PReLU(a) = a*x + relu((1-a)*x) via one ScalarE
    relu + one GpSimd scalar_tensor_tensor.
"""

import json
import sys

for _p in ("/opt/trn_rl_repo", "/root/.axon_site/_ro/trn_rl_repo"):
    if _p not in sys.path:
        sys.path.append(_p)

import numpy as np
import ml_dtypes

import concourse.bass as bass
import concourse.tile as tile
from concourse import mybir
from concourse.bass_utils import run_bass_kernel_spmd

B, CIN, L, COUT, K = 64, 64, 4096, 128, 7
PAD = 3
BN_EPS = 1e-5
N_CORES = 8
BPC = B // N_CORES  # samples per core
LOUT = L // 2       # 2048 pooled length
NT = L // 512       # 8 output tiles of 512 cols
XW = L + 8          # padded X2 width (4104)

_CACHE: dict = {}


def build_program() -> "bass.Bass":
    nc = bass.Bass(trn_type="TRN2")
    I8 = nc.dram_tensor("I8", [BPC, CIN, L], mybir.dt.float32, kind="ExternalInput")
    W = nc.dram_tensor("W", [128, K * 128], mybir.dt.bfloat16, kind="ExternalInput")
    SBp = nc.dram_tensor("SBp", [128, 4], mybir.dt.float32, kind="ExternalInput")
    O8 = nc.dram_tensor("O8", [BPC, COUT, LOUT], mybir.dt.bfloat16, kind="ExternalOutput")

    iflat = I8.ap().flatten_outer_dims()  # [BPC*64, 4096]
    oflat = O8.ap().flatten_outer_dims()  # [BPC*128, 2048]

    AF = mybir.ActivationFunctionType
    SGW = L + 8  # sg width: cols 0-2 zero pad, 3..L+2 data, L+3.. zero
    NHALF = NT // 2  # 4 l-tiles per half (A half + B half = 8 PSUM banks)
    with tile.TileContext(nc) as tc:
        with (
            tc.tile_pool(name="consts", bufs=1) as consts,
            tc.tile_pool(name="ipair", bufs=2) as ipool,
            tc.tile_pool(name="sgn", bufs=2) as spool,
            tc.tile_pool(name="pooled", bufs=2) as plpool,
            tc.tile_pool(name="outp", bufs=2) as opool,
            tc.tile_pool(name="ps", bufs=8, space="PSUM") as pspool,
        ):
            w_sb = consts.tile([128, K * 128], mybir.dt.bfloat16)
            nc.sync.dma_start(w_sb[:], W.ap()[:])
            sb_sb = consts.tile([128, 4], mybir.dt.float32)
            nc.sync.dma_start(sb_sb[:], SBp.ap()[:])
            sgn_scale = sb_sb[:, 0:1]
            sgn_bias = sb_sb[:, 1:2]
            slope = sb_sb[:, 3:4]  # a

            for t in range(BPC // 2):
                ip = ipool.tile([128, L], mybir.dt.float32)
                nc.sync.dma_start(ip[:], iflat[128 * t : 128 * (t + 1), :])
                # sign of BN affine for two samples at once; sg doubles as the
                # padded conv input (rows 0-63 = sample A, 64-127 = sample B)
                sg = spool.tile([128, SGW], mybir.dt.bfloat16)
                nc.gpsimd.memset(sg[:, 0:3], 0.0)
                nc.gpsimd.memset(sg[:, L + 3 : SGW], 0.0)
                nc.scalar.activation(
                    sg[:, 3 : L + 3], ip[:], AF.Sign, bias=sgn_bias, scale=sgn_scale
                )

                pla = plpool.tile([128, LOUT], mybir.dt.bfloat16, name="pla", tag="pla")
                plb = plpool.tile([128, LOUT], mybir.dt.bfloat16, name="plb", tag="plb")
                for it in range(NT):
                    psa = pspool.tile([128, 512], mybir.dt.float32, name="psa", tag="psb")
                    psb = pspool.tile([128, 512], mybir.dt.float32, name="psb", tag="psb")
                    for k in range(K):
                        c0 = 512 * it + k
                        nc.tensor.matmul(
                            psa[:], w_sb[0:64, 128 * k : 128 * (k + 1)],
                            sg[0:64, c0 : c0 + 512],
                            start=(k == 0), stop=(k == K - 1),
                        )
                        nc.tensor.matmul(
                            psb[:], w_sb[64:128, 128 * k : 128 * (k + 1)],
                            sg[64:128, c0 : c0 + 512],
                            start=(k == 0), stop=(k == K - 1),
                        )
                    o0 = 256 * it
                    nc.vector.tensor_reduce(
                        pla[:, o0 : o0 + 256],
                        psa[:].rearrange("p (n two) -> p n two", two=2),
                        mybir.AxisListType.X,
                        mybir.AluOpType.max,
                    )
                    nc.vector.tensor_reduce(
                        plb[:, o0 : o0 + 256],
                        psb[:].rearrange("p (n two) -> p n two", two=2),
                        mybir.AxisListType.X,
                        mybir.AluOpType.max,
                    )
                # prelu(x) = max(a*x, x) for 0 <= a <= 1
                for h, pl in ((0, pla), (1, plb)):
                    b = 2 * t + h
                    o = opool.tile([128, LOUT], mybir.dt.bfloat16)
                    nc.vector.scalar_tensor_tensor(
                        o[:], pl[:], slope, pl[:],
                        mybir.AluOpType.mult, mybir.AluOpType.max,
                    )
                    nc.sync.dma_start(oflat[128 * b : 128 * (b + 1), :], o[:])
    return nc


def _split_sync_waits_json(bir: bytes) -> bytes:
    """Walrus in this toolchain accepts at most one sync-wait per instruction.
    Hoist multi-wait sync_info lists into preceding single-wait EventSemaphore
    instructions on the same engine queue (the same form engine.wait_ge()
    lowers to), preserving program order and on_update placement."""
    j = json.loads(bir)
    n_split = 0
    for fn in j.get("functions", []):
        for blk in fn.get("blocks", []):
            ins_list = blk.get("instructions")
            if not ins_list:
                continue
            out = []
            for ins in ins_list:
                si = ins.get("sync_info")
                waits = si.get("on_wait") if si else None
                if waits and len(waits) > 1:
                    for i, w in enumerate(waits):
                        out.append(
                            {
                                "debug": ins.get("debug", 0),
                                "engine": ins["engine"],
                                "ins": [],
                                "outs": [],
                                "name": f"{ins['name']}-antw{i}",
                                "opcode": "EventSemaphore",
                                "sync_info": {"on_update": [], "on_wait": [w]},
                            }
                        )
                    si["on_wait"] = []
                    n_split += 1
                out.append(ins)
            blk["instructions"] = out
    return json.dumps(j).encode()


def get_program() -> "bass.Bass":
    if "nc" not in _CACHE:
        nc = build_program()
        orig = nc.to_json_bytes
        nc.to_json_bytes = lambda: _split_sync_waits_json(orig())
        _CACHE["nc"] = nc
    return _CACHE["nc"]


def prep_inputs(I, bn_gamma, bn_beta, bn_mean, bn_var, conv_w, alpha, prelu_w):
    """Host-side folding: BN -> (scale, bias); alpha -> weights; k-pair lhsT packing."""
    f32 = np.float32
    gamma = np.asarray(bn_gamma, f32)
    beta = np.asarray(bn_beta, f32)
    mean = np.asarray(bn_mean, f32)
    var = np.asarray(bn_var, f32)
    s = gamma / np.sqrt(var + f32(BN_EPS))        # [CIN]
    t = beta - mean * s                            # [CIN]

    w = np.asarray(conv_w, f32) * np.asarray(alpha, f32)[:, None, None]  # [COUT, CIN, K]
    Wb = np.zeros((128, K * 128), np.float32)
    for k in range(K):
        Wb[0:64, 128 * k : 128 * k + 128] = w[:, :, k].T
        Wb[64:128, 128 * k : 128 * k + 128] = w[:, :, k].T
    Wb = Wb.astype(ml_dtypes.bfloat16)

    a = f32(np.asarray(prelu_w, f32).reshape(-1)[0])
    sbp = np.zeros((128, 4), f32)
    sbp[0:64, 0] = s
    sbp[64:128, 0] = s
    sbp[0:64, 1] = t
    sbp[64:128, 1] = t
    sbp[:, 2] = f32(1.0) - a
    sbp[:, 3] = a
    return Wb, sbp


def kernel(I, bn_gamma, bn_beta, bn_mean, bn_var, conv_w, alpha, prelu_w):
    I = np.ascontiguousarray(np.asarray(I, np.float32))
    assert I.shape == (B, CIN, L), I.shape
    Wb, sbp = prep_inputs(I, bn_gamma, bn_beta, bn_mean, bn_var, conv_w, alpha, prelu_w)

    nc = get_program()
    in_maps = [
        {"I8": I[BPC * c : BPC * (c + 1)], "W": Wb, "SBp": sbp} for c in range(N_CORES)
    ]
    res = run_bass_kernel_spmd(nc, in_maps, core_ids=list(range(N_CORES)))
    out = np.concatenate(
        [np.asarray(res.results[c]["O8"]) for c in range(N_CORES)], axis=0
    )
    return np.ascontiguousarray(out.astype(np.float32))


# revision 15
# speedup vs baseline: 2.3061x; 1.1191x over previous
"""Trainium2 Bass kernel: BN(eval) -> sign -> Conv1d(K=7,pad=3) -> alpha -> PReLU -> MaxPool2.

Strategy (hardcoded for B=64, CIN=64, L=4096, COUT=128, K=7):
  - Data-parallel over batch: 8 samples per NeuronCore x 8 cores.
  - Host folds BN into per-channel (scale, bias); ScalarE computes
    sign(I*scale+bias) for a PAIR of samples at once ([128, 4096] -> bf16).
  - Host folds alpha into conv weights; conv done as 4 PSUM-accumulated
    bf16 matmuls per 512-col output tile: k-pairs (0,1),(2,3),(4,5),(6,-)
    packed into a 128-row contraction, with the input duplicated into
    partitions 64..127 shifted by one column.
  - MaxPool(2) straight out of PSUM via DVE tensor_reduce(max) over
    [128, 256, 2] views; PReLU(a) = a*x + relu((1-a)*x) via one ScalarE
    relu + one GpSimd scalar_tensor_tensor.
"""

import json
import sys

for _p in ("/opt/trn_rl_repo", "/root/.axon_site/_ro/trn_rl_repo"):
    if _p not in sys.path:
        sys.path.append(_p)

import numpy as np
import ml_dtypes

import concourse.bass as bass
import concourse.tile as tile
from concourse import mybir
from concourse.bass_utils import run_bass_kernel_spmd

B, CIN, L, COUT, K = 64, 64, 4096, 128, 7
PAD = 3
BN_EPS = 1e-5
N_CORES = 8
BPC = B // N_CORES  # samples per core
LOUT = L // 2       # 2048 pooled length
NT = L // 512       # 8 output tiles of 512 cols
XW = L + 8          # padded X2 width (4104)

_CACHE: dict = {}


def build_program(use_act_prelu: bool = True) -> "bass.Bass":
    nc = bass.Bass(trn_type="TRN2")
    I8 = nc.dram_tensor("I8", [BPC, CIN, L], mybir.dt.float32, kind="ExternalInput")
    W = nc.dram_tensor("W", [128, K * 128], mybir.dt.bfloat16, kind="ExternalInput")
    SBp = nc.dram_tensor("SBp", [128, 4], mybir.dt.float32, kind="ExternalInput")
    O8 = nc.dram_tensor("O8", [BPC, COUT, LOUT], mybir.dt.bfloat16, kind="ExternalOutput")

    iflat = I8.ap().flatten_outer_dims()  # [BPC*64, 4096]
    oflat = O8.ap().flatten_outer_dims()  # [BPC*128, 2048]

    AF = mybir.ActivationFunctionType
    SGW = L + 8  # sg width: cols 0-2 zero pad, 3..L+2 data, L+3.. zero
    NHALF = NT // 2  # 4 l-tiles per half (A half + B half = 8 PSUM banks)
    with tile.TileContext(nc) as tc:
        with (
            tc.tile_pool(name="consts", bufs=1) as consts,
            tc.tile_pool(name="ipair", bufs=2) as ipool,
            tc.tile_pool(name="sgn", bufs=2) as spool,
            tc.tile_pool(name="pooled", bufs=2) as plpool,
            tc.tile_pool(name="outp", bufs=2) as opool,
            tc.tile_pool(name="ps", bufs=8, space="PSUM") as pspool,
        ):
            w_sb = consts.tile([128, K * 128], mybir.dt.bfloat16)
            nc.sync.dma_start(w_sb[:], W.ap()[:])
            sb_sb = consts.tile([128, 4], mybir.dt.float32)
            nc.sync.dma_start(sb_sb[:], SBp.ap()[:])
            sgn_scale = sb_sb[:, 0:1]
            sgn_bias = sb_sb[:, 1:2]
            slope = sb_sb[:, 3:4]  # a

            NCHUNK = 2
            CW = L // NCHUNK
            for t in range(BPC // 2):
                # chunked input DMA + sign so the first matmuls start early
                ip = ipool.tile([128, L], mybir.dt.float32)
                sg = spool.tile([128, SGW], mybir.dt.bfloat16)
                nc.gpsimd.memset(sg[:, 0:3], 0.0)
                nc.gpsimd.memset(sg[:, L + 3 : SGW], 0.0)
                for c in range(NCHUNK):
                    nc.sync.dma_start(
                        ip[:, CW * c : CW * (c + 1)],
                        iflat[128 * t : 128 * (t + 1), CW * c : CW * (c + 1)],
                    )
                    nc.scalar.activation(
                        sg[:, 3 + CW * c : 3 + CW * (c + 1)],
                        ip[:, CW * c : CW * (c + 1)],
                        AF.Sign, bias=sgn_bias, scale=sgn_scale,
                    )

                pla = plpool.tile([128, LOUT], mybir.dt.bfloat16, name="pla", tag="pla")
                plb = plpool.tile([128, LOUT], mybir.dt.bfloat16, name="plb", tag="plb")
                for it in range(NT):
                    psa = pspool.tile([128, 512], mybir.dt.float32, name="psa", tag="psb")
                    psb = pspool.tile([128, 512], mybir.dt.float32, name="psb", tag="psb")
                    for k in range(K):
                        c0 = 512 * it + k
                        nc.tensor.matmul(
                            psa[:], w_sb[0:64, 128 * k : 128 * (k + 1)],
                            sg[0:64, c0 : c0 + 512],
                            start=(k == 0), stop=(k == K - 1),
                        )
                        nc.tensor.matmul(
                            psb[:], w_sb[64:128, 128 * k : 128 * (k + 1)],
                            sg[64:128, c0 : c0 + 512],
                            start=(k == 0), stop=(k == K - 1),
                        )
                    o0 = 256 * it
                    if use_act_prelu:
                        # prelu straight out of PSUM on ScalarE, then pairwise
                        # max-pool of the (monotone) prelu'd values on DVE
                        pra = opool.tile([128, 512], mybir.dt.bfloat16, name="pra", tag="pra")
                        nc.scalar.activation(pra[:], psa[:], AF.Prelu, alpha=slope)
                        prb = opool.tile([128, 512], mybir.dt.bfloat16, name="prb", tag="prb")
                        nc.scalar.activation(prb[:], psb[:], AF.Prelu, alpha=slope)
                        nc.vector.tensor_reduce(
                            pla[:, o0 : o0 + 256],
                            pra[:].rearrange("p (n two) -> p n two", two=2),
                            mybir.AxisListType.X,
                            mybir.AluOpType.max,
                        )
                        nc.vector.tensor_reduce(
                            plb[:, o0 : o0 + 256],
                            prb[:].rearrange("p (n two) -> p n two", two=2),
                            mybir.AxisListType.X,
                            mybir.AluOpType.max,
                        )
                    else:
                        nc.vector.tensor_reduce(
                            pla[:, o0 : o0 + 256],
                            psa[:].rearrange("p (n two) -> p n two", two=2),
                            mybir.AxisListType.X,
                            mybir.AluOpType.max,
                        )
                        nc.vector.tensor_reduce(
                            plb[:, o0 : o0 + 256],
                            psb[:].rearrange("p (n two) -> p n two", two=2),
                            mybir.AxisListType.X,
                            mybir.AluOpType.max,
                        )
                    if it == NT // 2 - 1 or it == NT - 1:
                        # flush each finished half to HBM
                        s0 = LOUT // 2 * (it // (NT // 2))
                        for h, pl in ((0, pla), (1, plb)):
                            b = 2 * t + h
                            if use_act_prelu:
                                nc.sync.dma_start(
                                    oflat[128 * b : 128 * (b + 1), s0 : s0 + LOUT // 2],
                                    pl[:, s0 : s0 + LOUT // 2],
                                )
                            else:
                                o = opool.tile(
                                    [128, LOUT // 2], mybir.dt.bfloat16, name="o", tag="o"
                                )
                                nc.vector.scalar_tensor_tensor(
                                    o[:], pl[:, s0 : s0 + LOUT // 2], slope,
                                    pl[:, s0 : s0 + LOUT // 2],
                                    mybir.AluOpType.mult, mybir.AluOpType.max,
                                )
                                nc.sync.dma_start(
                                    oflat[128 * b : 128 * (b + 1), s0 : s0 + LOUT // 2],
                                    o[:],
                                )
    return nc


def _split_sync_waits_json(bir: bytes) -> bytes:
    """Walrus in this toolchain accepts at most one sync-wait per instruction.
    Hoist multi-wait sync_info lists into preceding single-wait EventSemaphore
    instructions on the same engine queue (the same form engine.wait_ge()
    lowers to), preserving program order and on_update placement."""
    j = json.loads(bir)
    n_split = 0
    for fn in j.get("functions", []):
        for blk in fn.get("blocks", []):
            ins_list = blk.get("instructions")
            if not ins_list:
                continue
            out = []
            for ins in ins_list:
                si = ins.get("sync_info")
                waits = si.get("on_wait") if si else None
                if waits and len(waits) > 1:
                    for i, w in enumerate(waits):
                        out.append(
                            {
                                "debug": ins.get("debug", 0),
                                "engine": ins["engine"],
                                "ins": [],
                                "outs": [],
                                "name": f"{ins['name']}-antw{i}",
                                "opcode": "EventSemaphore",
                                "sync_info": {"on_update": [], "on_wait": [w]},
                            }
                        )
                    si["on_wait"] = []
                    n_split += 1
                out.append(ins)
            blk["instructions"] = out
    return json.dumps(j).encode()


def get_program() -> "bass.Bass":
    if "nc" not in _CACHE:
        nc = build_program()
        orig = nc.to_json_bytes
        nc.to_json_bytes = lambda: _split_sync_waits_json(orig())
        _CACHE["nc"] = nc
    return _CACHE["nc"]


def prep_inputs(I, bn_gamma, bn_beta, bn_mean, bn_var, conv_w, alpha, prelu_w):
    """Host-side folding: BN -> (scale, bias); alpha -> weights; k-pair lhsT packing."""
    f32 = np.float32
    gamma = np.asarray(bn_gamma, f32)
    beta = np.asarray(bn_beta, f32)
    mean = np.asarray(bn_mean, f32)
    var = np.asarray(bn_var, f32)
    s = gamma / np.sqrt(var + f32(BN_EPS))        # [CIN]
    t = beta - mean * s                            # [CIN]

    w = np.asarray(conv_w, f32) * np.asarray(alpha, f32)[:, None, None]  # [COUT, CIN, K]
    Wb = np.zeros((128, K * 128), np.float32)
    for k in range(K):
        Wb[0:64, 128 * k : 128 * k + 128] = w[:, :, k].T
        Wb[64:128, 128 * k : 128 * k + 128] = w[:, :, k].T
    Wb = Wb.astype(ml_dtypes.bfloat16)

    a = f32(np.asarray(prelu_w, f32).reshape(-1)[0])
    sbp = np.zeros((128, 4), f32)
    sbp[0:64, 0] = s
    sbp[64:128, 0] = s
    sbp[0:64, 1] = t
    sbp[64:128, 1] = t
    sbp[:, 2] = f32(1.0) - a
    sbp[:, 3] = a
    return Wb, sbp


def kernel(I, bn_gamma, bn_beta, bn_mean, bn_var, conv_w, alpha, prelu_w):
    I = np.ascontiguousarray(np.asarray(I, np.float32))
    assert I.shape == (B, CIN, L), I.shape
    Wb, sbp = prep_inputs(I, bn_gamma, bn_beta, bn_mean, bn_var, conv_w, alpha, prelu_w)

    nc = get_program()
    in_maps = [
        {"I8": I[BPC * c : BPC * (c + 1)], "W": Wb, "SBp": sbp} for c in range(N_CORES)
    ]
    res = run_bass_kernel_spmd(nc, in_maps, core_ids=list(range(N_CORES)))
    out = np.concatenate(
        [np.asarray(res.results[c]["O8"]) for c in range(N_CORES)], axis=0
    )
    return np.ascontiguousarray(out.astype(np.float32))


# revision 17
# speedup vs baseline: 2.4593x; 1.0664x over previous
"""Trainium2 Bass kernel: BN(eval) -> sign -> Conv1d(K=7,pad=3) -> alpha -> PReLU -> MaxPool2.

Strategy (hardcoded for B=64, CIN=64, L=4096, COUT=128, K=7):
  - Data-parallel over batch: 8 samples per NeuronCore x 8 cores.
  - Host folds BN into per-channel (scale, bias); ScalarE computes
    sign(I*scale+bias) for a PAIR of samples at once ([128, 4096] -> bf16).
  - Host folds alpha into conv weights; conv done as 4 PSUM-accumulated
    bf16 matmuls per 512-col output tile: k-pairs (0,1),(2,3),(4,5),(6,-)
    packed into a 128-row contraction, with the input duplicated into
    partitions 64..127 shifted by one column.
  - MaxPool(2) straight out of PSUM via DVE tensor_reduce(max) over
    [128, 256, 2] views; PReLU(a) = a*x + relu((1-a)*x) via one ScalarE
    relu + one GpSimd scalar_tensor_tensor.
"""

import json
import sys

for _p in ("/opt/trn_rl_repo", "/root/.axon_site/_ro/trn_rl_repo"):
    if _p not in sys.path:
        sys.path.append(_p)

import numpy as np
import ml_dtypes

import concourse.bass as bass
import concourse.tile as tile
from concourse import mybir
from concourse.bass_utils import run_bass_kernel_spmd

B, CIN, L, COUT, K = 64, 64, 4096, 128, 7
PAD = 3
BN_EPS = 1e-5
N_CORES = 8
BPC = B // N_CORES  # samples per core
LOUT = L // 2       # 2048 pooled length
NT = L // 512       # 8 output tiles of 512 cols
XW = L + 8          # padded X2 width (4104)

_CACHE: dict = {}


def build_program(use_act_prelu: bool = True) -> "bass.Bass":
    nc = bass.Bass(trn_type="TRN2")
    I8 = nc.dram_tensor("I8", [BPC, CIN, L], mybir.dt.float32, kind="ExternalInput")
    W = nc.dram_tensor("W", [128, K * 128], mybir.dt.bfloat16, kind="ExternalInput")
    SBp = nc.dram_tensor("SBp", [128, 4], mybir.dt.float32, kind="ExternalInput")
    O8 = nc.dram_tensor("O8", [BPC, COUT, LOUT], mybir.dt.bfloat16, kind="ExternalOutput")

    iflat = I8.ap().flatten_outer_dims()  # [BPC*64, 4096]
    oflat = O8.ap().flatten_outer_dims()  # [BPC*128, 2048]

    AF = mybir.ActivationFunctionType
    SGW = L + 8  # sg width: cols 0-2 zero pad, 3..L+2 data, L+3.. zero
    NHALF = NT // 2  # 4 l-tiles per half (A half + B half = 8 PSUM banks)
    with tile.TileContext(nc) as tc:
        with (
            tc.tile_pool(name="consts", bufs=1) as consts,
            tc.tile_pool(name="ipair", bufs=2) as ipool,
            tc.tile_pool(name="sgn", bufs=2) as spool,
            tc.tile_pool(name="pooled", bufs=2) as plpool,
            tc.tile_pool(name="outp", bufs=2) as opool,
            tc.tile_pool(name="ps", bufs=8, space="PSUM") as pspool,
        ):
            w_sb = consts.tile([128, K * 128], mybir.dt.bfloat16)
            nc.sync.dma_start(w_sb[:], W.ap()[:])
            sb_sb = consts.tile([128, 4], mybir.dt.float32)
            nc.sync.dma_start(sb_sb[:], SBp.ap()[:])
            sgn_scale = sb_sb[:, 0:1]
            sgn_bias = sb_sb[:, 1:2]
            slope = sb_sb[:, 3:4]  # a

            NCHUNK = 4
            CW = L // NCHUNK
            for t in range(BPC // 2):
                # chunked input DMA + sign so the first matmuls start early
                ip = ipool.tile([128, L], mybir.dt.float32)
                sg = spool.tile([128, SGW], mybir.dt.bfloat16)
                nc.gpsimd.memset(sg[:, 0:3], 0.0)
                nc.gpsimd.memset(sg[:, L + 3 : SGW], 0.0)
                for c in range(NCHUNK):
                    nc.sync.dma_start(
                        ip[:, CW * c : CW * (c + 1)],
                        iflat[128 * t : 128 * (t + 1), CW * c : CW * (c + 1)],
                    )
                    nc.scalar.activation(
                        sg[:, 3 + CW * c : 3 + CW * (c + 1)],
                        ip[:, CW * c : CW * (c + 1)],
                        AF.Sign, bias=sgn_bias, scale=sgn_scale,
                    )

                pla = plpool.tile([128, LOUT], mybir.dt.bfloat16, name="pla", tag="pla")
                plb = plpool.tile([128, LOUT], mybir.dt.bfloat16, name="plb", tag="plb")
                for it in range(NT):
                    psa = pspool.tile([128, 512], mybir.dt.float32, name="psa", tag="psb")
                    psb = pspool.tile([128, 512], mybir.dt.float32, name="psb", tag="psb")
                    for k in range(K):
                        c0 = 512 * it + k
                        nc.tensor.matmul(
                            psa[:], w_sb[0:64, 128 * k : 128 * (k + 1)],
                            sg[0:64, c0 : c0 + 512],
                            start=(k == 0), stop=(k == K - 1),
                        )
                        nc.tensor.matmul(
                            psb[:], w_sb[64:128, 128 * k : 128 * (k + 1)],
                            sg[64:128, c0 : c0 + 512],
                            start=(k == 0), stop=(k == K - 1),
                        )
                    o0 = 256 * it
                    nc.vector.tensor_reduce(
                        pla[:, o0 : o0 + 256],
                        psa[:].rearrange("p (n two) -> p n two", two=2),
                        mybir.AxisListType.X,
                        mybir.AluOpType.max,
                    )
                    nc.vector.tensor_reduce(
                        plb[:, o0 : o0 + 256],
                        psb[:].rearrange("p (n two) -> p n two", two=2),
                        mybir.AxisListType.X,
                        mybir.AluOpType.max,
                    )
                    if it == NT // 2 - 1 or it == NT - 1:
                        # prelu on the pooled half (prelu commutes with max),
                        # then flush it to HBM
                        s0 = LOUT // 2 * (it // (NT // 2))
                        for h, pl in ((0, pla), (1, plb)):
                            b = 2 * t + h
                            o = opool.tile(
                                [128, LOUT // 2], mybir.dt.bfloat16, name="o", tag="o"
                            )
                            if use_act_prelu:
                                nc.scalar.activation(
                                    o[:], pl[:, s0 : s0 + LOUT // 2], AF.Prelu,
                                    alpha=slope,
                                )
                            else:
                                nc.vector.scalar_tensor_tensor(
                                    o[:], pl[:, s0 : s0 + LOUT // 2], slope,
                                    pl[:, s0 : s0 + LOUT // 2],
                                    mybir.AluOpType.mult, mybir.AluOpType.max,
                                )
                            nc.sync.dma_start(
                                oflat[128 * b : 128 * (b + 1), s0 : s0 + LOUT // 2],
                                o[:],
                            )
    return nc


def _split_sync_waits_json(bir: bytes) -> bytes:
    """Walrus in this toolchain accepts at most one sync-wait per instruction.
    Hoist multi-wait sync_info lists into preceding single-wait EventSemaphore
    instructions on the same engine queue (the same form engine.wait_ge()
    lowers to), preserving program order and on_update placement."""
    j = json.loads(bir)
    n_split = 0
    for fn in j.get("functions", []):
        for blk in fn.get("blocks", []):
            ins_list = blk.get("instructions")
            if not ins_list:
                continue
            out = []
            for ins in ins_list:
                si = ins.get("sync_info")
                waits = si.get("on_wait") if si else None
                if waits and len(waits) > 1:
                    for i, w in enumerate(waits):
                        out.append(
                            {
                                "debug": ins.get("debug", 0),
                                "engine": ins["engine"],
                                "ins": [],
                                "outs": [],
                                "name": f"{ins['name']}-antw{i}",
                                "opcode": "EventSemaphore",
                                "sync_info": {"on_update": [], "on_wait": [w]},
                            }
                        )
                    si["on_wait"] = []
                    n_split += 1
                out.append(ins)
            blk["instructions"] = out
    return json.dumps(j).encode()


def get_program() -> "bass.Bass":
    if "nc" not in _CACHE:
        nc = build_program()
        orig = nc.to_json_bytes
        nc.to_json_bytes = lambda: _split_sync_waits_json(orig())
        _CACHE["nc"] = nc
    return _CACHE["nc"]


def prep_inputs(I, bn_gamma, bn_beta, bn_mean, bn_var, conv_w, alpha, prelu_w):
    """Host-side folding: BN -> (scale, bias); alpha -> weights; k-pair lhsT packing."""
    f32 = np.float32
    gamma = np.asarray(bn_gamma, f32)
    beta = np.asarray(bn_beta, f32)
    mean = np.asarray(bn_mean, f32)
    var = np.asarray(bn_var, f32)
    s = gamma / np.sqrt(var + f32(BN_EPS))        # [CIN]
    t = beta - mean * s                            # [CIN]

    w = np.asarray(conv_w, f32) * np.asarray(alpha, f32)[:, None, None]  # [COUT, CIN, K]
    Wb = np.zeros((128, K * 128), np.float32)
    for k in range(K):
        Wb[0:64, 128 * k : 128 * k + 128] = w[:, :, k].T
        Wb[64:128, 128 * k : 128 * k + 128] = w[:, :, k].T
    Wb = Wb.astype(ml_dtypes.bfloat16)

    a = f32(np.asarray(prelu_w, f32).reshape(-1)[0])
    sbp = np.zeros((128, 4), f32)
    sbp[0:64, 0] = s
    sbp[64:128, 0] = s
    sbp[0:64, 1] = t
    sbp[64:128, 1] = t
    sbp[:, 2] = f32(1.0) - a
    sbp[:, 3] = a
    return Wb, sbp


def kernel(I, bn_gamma, bn_beta, bn_mean, bn_var, conv_w, alpha, prelu_w):
    I = np.ascontiguousarray(np.asarray(I, np.float32))
    assert I.shape == (B, CIN, L), I.shape
    Wb, sbp = prep_inputs(I, bn_gamma, bn_beta, bn_mean, bn_var, conv_w, alpha, prelu_w)

    nc = get_program()
    in_maps = [
        {"I8": I[BPC * c : BPC * (c + 1)], "W": Wb, "SBp": sbp} for c in range(N_CORES)
    ]
    res = run_bass_kernel_spmd(nc, in_maps, core_ids=list(range(N_CORES)))
    out = np.concatenate(
        [np.asarray(res.results[c]["O8"]) for c in range(N_CORES)], axis=0
    )
    return np.ascontiguousarray(out.astype(np.float32))
